# revision 6
# baseline (speedup 1.0000x reference)
"""Expected Calibration Error (histogram binning) on 8 Trainium2 NeuronCores.

kernel(outputs [1e6,100] f32, targets [1e6] int) -> f32 scalar, matching the
reference softmax/argmax/10-bin ECE. Data-parallel over the batch; each core
streams its shard once from HBM in float16 (~25 MB/core, ~70 us at 360 GB/s).

Layout trick: every per-row ECE statistic (row max, row sum of exp, true-class
prob, argmax==target, bin membership) is invariant to a cyclic rotation of the
row's class axis. The host rolls each row left by its target class — the
true-class logit lands in column 0 for every row — casts to f16 (measured ECE
error 6e-3 relative, vs the 2e-2 gate), and packs rows densely onto a
[8 cores, 128 partitions, 980 rows] grid (0.35% padding; pad rows are
[-300, 0, ...], whose exp underflows to exactly 0 so p == 0 and the
reference's own p > 0 rule excludes them). The true-class logit column is also
shipped separately as xt [P, W] so no strided on-chip extraction is needed.

Device, per chunk of 49 rows/partition ([128, 49, 100] f16 tile):
  - ACT:  exp of the whole tile (f16 out) — the only engine with exp; at
          1 elem/cycle/partition this ~82 us is the kernel's floor.
  - PE:   row-sum-of-exp via 100 accumulating f16 identity matmuls
          (1 cycle/row vs 4 for f32) into PSUM f32.
  - DVE:  row max via a pairwise f16 max tree (tensor_tensor supports the
          2x_1p DVE mode = 0.52 ns/elem; tensor_reduce gets no fast mode).
          Level 2 of the tree runs on the otherwise-idle GPSIMD.
  - DVE:  reciprocal PSUM->SBUF per chunk frees the PSUM buffer early.
Finish (interleaved slabs): p16 = exp(x_t) * (1/S) rounded to f16,
correct = (x_t == m), z16 = p16 * correct (GPSIMD); then per boundary b of
the f16-snapped linspace(0,1,11) grid, single-instruction DVE scans in the
4x_2p mode (0.26 ns/elem) with the free-axis accumulator:
  C_b = count(p16 > b)          (op0=is_gt, accum add)
  K_b = sum(max(p16, b))        (op0=max,   accum add)
  Z_b = count(z16 > b)          (op0=is_gt, accum add)
Host: sum the 8x128 partials in f64; SP_b = K_b - b*(rows - C_b) recovers the
cumulative sum of confidences; adjacent differences give the 10 bins; finish
the ECE scalar exactly as the reference does. Boundaries are exact f16 grid
points so device-side f16 rounding of the scalar is a no-op.
"""

import os
import sys
import tempfile

import numpy as np

if "/opt/trn_rl_repo" not in sys.path:
    sys.path.insert(0, "/opt/trn_rl_repo")

# Persistent jax/PJRT executable cache (includes the compiled NEFF): makes
# repeat invocations and the subprocess-retry path skip the ~60s neuronx
# compile. Must be set before jax initializes.
os.environ.setdefault(
    "JAX_COMPILATION_CACHE_DIR",
    os.path.join(tempfile.gettempdir(), "jaxcache"),
)

N = 1_000_000
C = 100
NCORES = 8
P = 128
W = 980
G = 49
CHUNKS = W // G      # 20
_SLAB_ENDS = [int(v) for v in os.environ.get("KV_SLABS", "15,20").split(",")]
NSLAB = len(_SLAB_ENDS)
XBUFS = int(os.environ.get("KV_XBUFS", "3"))
EBUFS = int(os.environ.get("KV_EBUFS", "2"))
PSBUFS = int(os.environ.get("KV_PSBUFS", "3"))
NPAD = NCORES * P * W
SENT = -300.0

# Bin boundaries snapped to the f16 grid (exactly representable in f16 AND
# f32) so the device compare/max against them is rounding-proof.
_BOUNDS = np.float32(np.float16(np.linspace(0.0, 1.0, 11).astype(np.float32)))

_built = {}


def _build_program():
    if "nc" in _built:
        return _built["nc"]

    import concourse.bacc as bacc
    import concourse.tile as tile
    from concourse import mybir

    f32 = mybir.dt.float32
    f16 = mybir.dt.float16
    Alu = mybir.AluOpType
    Act = mybir.ActivationFunctionType

    nc = bacc.Bacc("TRN2", target_bir_lowering=False, debug=False)
    x_d = nc.dram_tensor("x", [P, W * C], f16, kind="ExternalInput").ap()
    xt_d = nc.dram_tensor("xt", [P, W], f16, kind="ExternalInput").ap()
    ident_d = nc.dram_tensor("ident", [P, P], f16, kind="ExternalInput").ap()
    acc_d = nc.dram_tensor("acc", [P, 33 * NSLAB], f32, kind="ExternalOutput").ap()

    slab_cols = [0] + [e * G for e in _SLAB_ENDS]
    assert slab_cols[-1] == W
    MAXSLAB = max(b - a for a, b in zip(slab_cols, slab_cols[1:]))

    # pairwise max-tree level widths over the class axis; overlapping windows
    # are safe because max is idempotent.
    LV = [50, 25, 13, 7, 4, 2]

    with tile.TileContext(nc) as tc:
        with (
            tc.tile_pool(name="consts", bufs=1) as consts,
            tc.tile_pool(name="stats", bufs=1) as stats,
            tc.tile_pool(name="xin", bufs=XBUFS) as xin,
            tc.tile_pool(name="etmp", bufs=EBUFS) as etmp,
            tc.tile_pool(name="tree", bufs=2) as tree,
            tc.tile_pool(name="psum", bufs=PSBUFS, space="PSUM") as psp,
        ):
            ident_t = consts.tile([P, P], f16)
            nc.gpsimd.dma_start(ident_t[:], ident_d[:, :])

            M = stats.tile([P, W], f16, tag="M")
            XT = stats.tile([P, W], f16, tag="XT")
            RS = stats.tile([P, W], f32, tag="RS")
            PTn = stats.tile([P, W], f32, tag="PTn")
            corr = stats.tile([P, W], f16, tag="corr")
            p16 = stats.tile([P, W], f16, tag="p16")
            z16 = stats.tile([P, W], f16, tag="z16")
            ACC = stats.tile([P, 33 * NSLAB], f32, tag="ACC")
            junk = stats.tile([P, MAXSLAB], f16, tag="junk")

            nc.sync.dma_start(XT[:], xt_d[:, :])

            def finish_slab(si):
                c0, c1 = slab_cols[si], slab_cols[si + 1]
                nw = c1 - c0
                nc.scalar.activation(PTn[:, c0:c1], XT[:, c0:c1], Act.Exp)
                nc.vector.tensor_tensor(
                    corr[:, c0:c1], XT[:, c0:c1], M[:, c0:c1], op=Alu.is_equal
                )
                nc.vector.tensor_tensor(
                    p16[:, c0:c1], PTn[:, c0:c1], RS[:, c0:c1], op=Alu.mult
                )
                nc.gpsimd.tensor_tensor(
                    z16[:, c0:c1], p16[:, c0:c1], corr[:, c0:c1], op=Alu.mult
                )
                ab = 33 * si
                # C/K scans need only p16; Z scans wait for Pool's z16 — put
                # them last so they overlap.
                for b in range(11):
                    lo = float(_BOUNDS[b])
                    nc.vector.tensor_scalar(
                        junk[:, :nw], p16[:, c0:c1], lo, None,
                        op0=Alu.is_gt, op1=Alu.add,
                        accum_out=ACC[:, ab + b:ab + b + 1],
                    )
                    nc.vector.tensor_scalar(
                        junk[:, :nw], p16[:, c0:c1], lo, None,
                        op0=Alu.max, op1=Alu.add,
                        accum_out=ACC[:, ab + 11 + b:ab + 12 + b],
                    )
                for b in range(11):
                    lo = float(_BOUNDS[b])
                    nc.vector.tensor_scalar(
                        junk[:, :nw], z16[:, c0:c1], lo, None,
                        op0=Alu.is_gt, op1=Alu.add,
                        accum_out=ACC[:, ab + 22 + b:ab + 23 + b],
                    )

            for k in range(CHUNKS):
                X = xin.tile([P, G * C], f16)
                nc.sync.dma_start(X[:], x_d[:, k * G * C:(k + 1) * G * C])
                x3 = X[:].rearrange("p (g c) -> p g c", c=C)

                E = etmp.tile([P, G * C], f16)
                nc.scalar.activation(E[:], X[:], Act.Exp)
                e3 = E[:].rearrange("p (g c) -> p g c", c=C)

                PS = psp.tile([P, G], f32)
                for cc in range(C):
                    nc.tensor.matmul(
                        PS[:], ident_t[:],
                        e3[:, :, cc:cc + 1].rearrange("p g c -> p (g c)"),
                        start=(cc == 0), stop=(cc == C - 1),
                    )

                # pairwise max tree over classes; level 2 on GPSIMD.
                # Emitted before the reciprocal so DVE's in-order stream isn't
                # stalled behind the PE matmul chain (the tree frees X, which
                # gates the next DMA).
                t3 = []
                for j, w in enumerate(LV):
                    tl = tree.tile([P, G * w], f16, tag=f"t{j}", name=f"t{j}_{k}")
                    t3.append(tl[:].rearrange("p (g c) -> p g c", c=w))
                nc.vector.tensor_tensor(
                    t3[0], x3[:, :, 0:50], x3[:, :, 50:100], op=Alu.max
                )
                nc.gpsimd.tensor_tensor(
                    t3[1], t3[0][:, :, 0:25], t3[0][:, :, 25:50], op=Alu.max
                )
                nc.vector.tensor_tensor(
                    t3[2], t3[1][:, :, 0:13], t3[1][:, :, 12:25], op=Alu.max
                )
                nc.vector.tensor_tensor(
                    t3[3], t3[2][:, :, 0:7], t3[2][:, :, 6:13], op=Alu.max
                )
                nc.vector.tensor_tensor(
                    t3[4], t3[3][:, :, 0:4], t3[3][:, :, 3:7], op=Alu.max
                )
                nc.vector.tensor_tensor(
                    t3[5], t3[4][:, :, 0:2], t3[4][:, :, 2:4], op=Alu.max
                )
                nc.vector.tensor_tensor(
                    M[:, k * G:(k + 1) * G],
                    t3[5][:, :, 0:1].rearrange("p g c -> p (g c)"),
                    t3[5][:, :, 1:2].rearrange("p g c -> p (g c)"),
                    op=Alu.max,
                )
                nc.vector.reciprocal(RS[:, k * G:(k + 1) * G], PS[:])

                if (k + 1) in _SLAB_ENDS:
                    finish_slab(_SLAB_ENDS.index(k + 1))

            nc.sync.dma_start(acc_d[:, :], ACC[:])

    nc.compile()
    _built["nc"] = nc
    return nc


def _prep_inputs(outputs, targets):
    """Sort rows by class, roll each row left by its class, pack densely,
    cast to f16, and extract the true-class column."""
    x = np.asarray(outputs, dtype=np.float32).astype(np.float16)
    t = np.asarray(targets).astype(np.int64).ravel()
    order = np.argsort(t, kind="stable")
    cnt = np.bincount(t, minlength=C)
    starts = np.zeros(C + 1, np.int64)
    starts[1:] = np.cumsum(cnt)

    Xr = np.empty((NPAD, C), np.float16)
    for c in range(C):
        s0, s1 = starts[c], starts[c + 1]
        if s1 == s0:
            continue
        src = x[order[s0:s1]]
        Xr[s0:s1, :C - c] = src[:, c:]
        if c:
            Xr[s0:s1, C - c:] = src[:, :c]
    Xr[N:] = 0.0
    Xr[N:, 0] = SENT

    xt = np.ascontiguousarray(Xr[:, 0]).reshape(NCORES, P, W)
    Xv = Xr.reshape(NCORES, P, W * C)
    ident = np.eye(P, dtype=np.float16)
    return [
        {"x": Xv[c], "xt": xt[c], "ident": ident} for c in range(NCORES)
    ]


def _postprocess(acc_list):
    A = np.stack(acc_list)
    tot = A.astype(np.float64).sum(axis=(0, 1))
    tot = tot.reshape(NSLAB, 33).sum(axis=0)
    Cg, K, Zg = tot[0:11], tot[11:22], tot[22:33]
    bounds = _BOUNDS.astype(np.float64)
    SPcum = K - bounds * (NPAD - Cg)          # sum of p over {p > bound[b]}
    cnt = Cg[:10] - Cg[1:]
    sp = SPcum[:10] - SPcum[1:]
    sc = Zg[:10] - Zg[1:]
    nonempty = cnt > 0
    denom = np.where(nonempty, cnt, 1.0)
    ece = np.sum(np.where(nonempty, cnt * np.abs(sp / denom - sc / denom), 0.0))
    total = cnt.sum()
    val = ece / max(total, 1.0) if total > 0 else 0.0
    return np.float32(val)


def _exec(in_maps, trace=False):
    from concourse.bass_utils import run_bass_kernel_spmd

    nc = _build_program()
    res = run_bass_kernel_spmd(
        nc, in_maps, core_ids=list(range(NCORES)), trace=trace
    )
    return [res.results[c]["acc"] for c in range(NCORES)], res


def _subrun(tmpdir):
    """Subprocess entry: load prepped inputs, execute, save partials."""
    in_maps = []
    for c in range(NCORES):
        in_maps.append({
            "x": np.load(f"{tmpdir}/x{c}.npy"),
            "xt": np.load(f"{tmpdir}/xt{c}.npy"),
            "ident": np.load(f"{tmpdir}/ident.npy"),
        })
    accs, _ = _exec(in_maps)
    np.save(f"{tmpdir}/accs.npy", np.stack(accs))


def _exec_subprocess(in_maps):
    """Run the device step in a fresh process (fresh PJRT client) — recovers
    from transient 'accelerator device unrecoverable' states."""
    import subprocess
    import tempfile

    here = os.path.dirname(os.path.abspath(__file__))
    with tempfile.TemporaryDirectory() as td:
        for c in range(NCORES):
            np.save(f"{td}/x{c}.npy", in_maps[c]["x"])
            np.save(f"{td}/xt{c}.npy", in_maps[c]["xt"])
        np.save(f"{td}/ident.npy", in_maps[0]["ident"])
        code = (
            f"import sys; sys.path.insert(0, {here!r}); "
            f"import kernel; kernel._subrun({td!r})"
        )
        subprocess.run([sys.executable, "-c", code], check=True, timeout=2400)
        accs = np.load(f"{td}/accs.npy")
    return [accs[c] for c in range(NCORES)]


def _run(outputs, targets, trace=False):
    import time

    in_maps = _prep_inputs(outputs, targets)
    accs = None
    last_err = None
    try:
        accs, res = _exec(in_maps, trace=trace)
    except Exception as e:  # transient device-unrecoverable errors
        last_err = e
        res = None
        sys.stderr.write(f"kernel: in-process exec failed: {e}\n")
    if accs is None:
        for attempt in range(3):
            try:
                time.sleep(5.0)
                accs = _exec_subprocess(in_maps)
                break
            except Exception as e:
                last_err = e
                sys.stderr.write(
                    f"kernel: subprocess exec attempt {attempt} failed: {e}\n"
                )
        else:
            raise last_err
    val = _postprocess(accs)
    return val, res


def kernel(outputs, targets):
    val, _ = _run(outputs, targets, trace=False)
    return val
